# revision 10
# baseline (speedup 1.0000x reference)
"""CNF2 (continuous normalizing flow step) Trainium2 kernel, 8-core data-parallel.

reference:
    p  = fc3(tanh(fc1(t)))                        # hypernet -> W,U,B (tiny)
    h  = tanh(W @ z.T + B)                        # [128, B]
    dz = (h.T @ U) / 128                          # [B, 64]
    dlogp = -(1/128) * (1 - h*h).T @ rowsum(W*U)  # [B, 1]

Plan:
  stage A (jax shard_map, 8 cores): hypernet with fc3 row-sharded +
    all_gather, then derived replicated params (W.T x2 bf16, B,
    U/128 bf16, s/128 bf16, -sum(s/128)).  0.2% of FLOPs.
  stage B (bass, 8 cores SPMD): batch sharded 32768 rows/core.
    Per 4096-row block: SWDGE cast-DMA loads z as bf16 [128, 2048]
    (partition p = 32 contiguous rows -> fully contiguous reads), one
    xbar DMA-transpose instruction yields all 16 z.T tiles ([128,16,128]
    batched form), h-matmul with W.T stationary (two N=512 bf16 matmuls
    per 1024-batch superstep), tanh on ACT (bias fused, bf16 out),
    h*h split DVE/GPSIMD, dz/trace matmuls with per-tile h/hh stationary
    (bf16 -> fast weight load), dz/dlogp staged in SBUF so stores are
    again fully contiguous 1MB DMAs.  The batch permutation introduced
    by the transpose tiling cancels between load and store layouts.
"""
import os
import sys
import time
from contextlib import ExitStack

for _p in ("/opt/trn_rl_repo", "/root/.axon_site/_ro/trn_rl_repo"):
    if os.path.isdir(_p) and _p not in sys.path:
        sys.path.insert(0, _p)

import numpy as np

IN_OUT_DIM = 64
HIDDEN_DIM = 512
WIDTH = 128
BATCH = 262144
BLOCKSIZE = WIDTH * IN_OUT_DIM
N_CORES = 8
BC = BATCH // N_CORES          # 32768 rows per core
BLK = 2048                     # batch rows per block
N_BLOCKS = BC // BLK           # 16
TPB = BLK // 128               # 16 transpose tiles per block
N_SS = 2                       # supersteps per block (8 tiles each)
LOOKAHEAD = 3                  # blocks of z load/transpose ahead of compute
# tile order produced by the two wide h-matmuls (even halves then odd)
SIGMA = (0, 2, 4, 6, 1, 3, 5, 7)

_CACHE = {}


def _enable_jax_cache():
    try:
        import jax
        d = "/tmp/jax_cache_cnf2"
        os.makedirs(d, exist_ok=True)
        jax.config.update("jax_compilation_cache_dir", d)
        jax.config.update("jax_persistent_cache_min_entry_size_bytes", -1)
        jax.config.update("jax_persistent_cache_min_compile_time_secs", 0)
    except Exception:
        pass


# ---------------------------------------------------------------------------
# workarounds: this walrus build accepts at most ONE sync-wait per
# instruction on CTRL templates; Tile's tail drain attaches one per live
# semaphore.  Split extra waits onto same-engine NoOps (same-engine program
# order preserves the wait-before-execute semantics).
# ---------------------------------------------------------------------------

def _install_tile_patch():
    import concourse.tile as tile
    from concourse import mybir
    from concourse.vector_clock import ScopedClock

    def _patched(self, tick_clock, wait_clock):
        drain_inst = self.nc.sync.drain()
        wait_clock.add_sem_waits(
            drain_inst.ins, ScopedClock({None: tick_clock.global_clock})
        )
        si = drain_inst.ins.sync_info
        waits = list(si.on_wait or []) if si is not None else []
        if len(waits) > 1:
            si.on_wait = waits[:1]
            for w in waits[1:]:
                d2 = self.nc.sync.drain()
                if d2.ins.sync_info is None:
                    d2.ins.sync_info = mybir.SyncInfo(on_wait=[w], on_update=[])
                else:
                    d2.ins.sync_info.on_wait = [w]
        self.nc.all_engine_barrier()
        assert self.sems is not None
        popped = self.nc._tile_sem_poison_stack.pop()
        assert popped is self._sem_poison
        self.nc.clear_and_free_semaphores(list(self.sems.allocated().values()))
        self.nc.all_engine_barrier()

    tile.TileContext._drain_and_barrier = _patched


def _fix_ctrl_waits(nc, max_waits=1):
    from concourse import mybir
    n_fixed = 0
    for fn in nc.m.functions:
        for bb in fn.blocks:
            out = []
            for inst in bb.instructions:
                si = inst.sync_info
                waits = list(si.on_wait) if (si is not None and si.on_wait) else []
                if len(waits) > max_waits:
                    keep, rest = waits[:max_waits], waits[max_waits:]
                    k = 0
                    while rest:
                        chunk, rest = rest[:max_waits], rest[max_waits:]
                        nop = mybir.InstNoOp(name=f"{inst.name}-xw{k}")
                        nop.engine = inst.engine
                        nop.sync_info = mybir.SyncInfo(on_wait=chunk, on_update=[])
                        out.append(nop)
                        k += 1
                    si.on_wait = keep
                    n_fixed += 1
                out.append(inst)
            bb.instructions = out
    return n_fixed


# ---------------------------------------------------------------------------
# stage B: the main bass kernel
# ---------------------------------------------------------------------------

def _build_bass():
    import concourse.bass as bass
    import concourse.tile as tile
    from concourse import mybir

    _install_tile_patch()

    f32 = mybir.dt.float32
    bf16 = mybir.dt.bfloat16
    nc = bass.Bass(num_devices=N_CORES)

    z = nc.declare_dram_parameter("z", [BC, IN_OUT_DIM], f32, isOutput=False)
    wt2 = nc.declare_dram_parameter("wt2", [128, 128], bf16, isOutput=False)
    bias = nc.declare_dram_parameter("bias", [128, 1], f32, isOutput=False)
    ub = nc.declare_dram_parameter("ub", [128, 64], bf16, isOutput=False)
    sb = nc.declare_dram_parameter("sb", [128, 1], bf16, isOutput=False)
    neg_s0 = nc.declare_dram_parameter("neg_s0", [128, 1], f32, isOutput=False)
    dz = nc.declare_dram_parameter("dz", [BC, IN_OUT_DIM], f32, isOutput=True)
    dlogp = nc.declare_dram_parameter("dlogp", [BC, 1], f32, isOutput=True)

    Tanh = mybir.ActivationFunctionType.Tanh

    with tile.TileContext(nc) as tc:
        with ExitStack() as ctx:
            cpool = ctx.enter_context(tc.tile_pool(name="consts", bufs=1))
            zpool = ctx.enter_context(tc.tile_pool(name="zbuf", bufs=LOOKAHEAD + 1))
            ztpool = ctx.enter_context(tc.tile_pool(name="zt", bufs=LOOKAHEAD + 1))
            h_pool = ctx.enter_context(tc.tile_pool(name="h", bufs=2))
            hh_pool = ctx.enter_context(tc.tile_pool(name="hh", bufs=2))
            dzs_pool = ctx.enter_context(tc.tile_pool(name="dzs", bufs=2))
            dlp_pool = ctx.enter_context(tc.tile_pool(name="dlp", bufs=1))
            ps_hp = ctx.enter_context(tc.tile_pool(name="ps_hp", bufs=2, space="PSUM"))
            ps_dz = ctx.enter_context(tc.tile_pool(name="ps_dz", bufs=2, space="PSUM"))
            ps_tr = ctx.enter_context(tc.tile_pool(name="ps_tr", bufs=2, space="PSUM"))

            # replicated constants
            wt2_t = cpool.tile([128, 128], bf16)
            nc.sync.dma_start(wt2_t[:], wt2[:])
            ub_t = cpool.tile([128, 64], bf16)
            nc.sync.dma_start(ub_t[:], ub[:])
            sb_t = cpool.tile([128, 1], bf16)
            nc.sync.dma_start(sb_t[:], sb[:])
            bias_t = cpool.tile([128, 1], f32)
            nc.sync.dma_start(bias_t[:], bias[:])
            ns0_t = cpool.tile([128, 1], f32)
            nc.sync.dma_start(ns0_t[:], neg_s0[:])

            dlp_stage = dlp_pool.tile([128, N_BLOCKS * TPB], f32)

            def issue_load(blk):
                """SWDGE cast-load z block -> bf16, then one xbar transpose
                instruction for all its z.T pair-tiles."""
                zv = z[blk * BLK:(blk + 1) * BLK, :].rearrange(
                    "(p r) d -> p (r d)", p=128
                )
                zbuf = zpool.tile([128, TPB * IN_OUT_DIM], bf16, tag="zbuf")
                nc.gpsimd.dma_start(zbuf[:], zv)
                zt = ztpool.tile([128, TPB * IN_OUT_DIM], bf16, tag="zt")
                nc.sync.dma_start(
                    zt[:].rearrange("p (j b) -> p j b", j=TPB // 2),
                    zbuf[:],
                    transpose=True,
                )
                return zt

            zts = {}
            for blk in range(LOOKAHEAD):
                zts[blk] = issue_load(blk)

            for blk in range(N_BLOCKS):
                if blk + LOOKAHEAD < N_BLOCKS:
                    zts[blk + LOOKAHEAD] = issue_load(blk + LOOKAHEAD)
                zt = zts.pop(blk)
                dz_stage = dzs_pool.tile([128, TPB * IN_OUT_DIM], f32, tag="dzs")
                tr_ps = ps_tr.tile([128, TPB], f32, tag="tr")

                for ss in range(N_SS):
                    t0 = ss * 8
                    # --- h matmuls: two wide N=512 (even tiles then odd)
                    hp_ps = ps_hp.tile([128, 1024], f32, tag="hp")
                    nc.tensor.matmul(
                        hp_ps[:, 0:512], wt2_t[0:64, :],
                        zt[0:64, ss * 512:(ss + 1) * 512],
                    )
                    nc.tensor.matmul(
                        hp_ps[:, 512:1024], wt2_t[64:128, :],
                        zt[64:128, ss * 512:(ss + 1) * 512],
                    )

                    # --- tanh (+bias) -> bf16 h
                    h_sb = h_pool.tile([128, 1024], bf16, tag="h")
                    nc.scalar.activation(h_sb[:], hp_ps[:], Tanh, bias=bias_t[:])

                    # --- hh = h*h split between gpsimd and DVE
                    hh_sb = hh_pool.tile([128, 1024], bf16, tag="hh")
                    nc.gpsimd.tensor_mul(
                        hh_sb[:, 0:512], h_sb[:, 0:512], h_sb[:, 0:512]
                    )
                    nc.vector.tensor_mul(
                        hh_sb[:, 512:1024], h_sb[:, 512:1024], h_sb[:, 512:1024]
                    )

                    # --- dz + trace matmuls per 128-batch tile
                    dz_ps = ps_dz.tile([128, 512], f32, tag="dzp")
                    for k in range(8):
                        lh = h_sb[:, k * 128:(k + 1) * 128]
                        nc.tensor.matmul(
                            dz_ps[:, SIGMA[k] * 64:SIGMA[k] * 64 + 64], lh, ub_t[:]
                        )
                    for k in range(8):
                        t = t0 + SIGMA[k]
                        lhh = hh_sb[:, k * 128:(k + 1) * 128]
                        nc.tensor.matmul(tr_ps[:, t:t + 1], lhh, sb_t[:])

                    nc.vector.tensor_copy(
                        dz_stage[:, ss * 512:(ss + 1) * 512], dz_ps[:]
                    )

                # dlogp for the block: add -sum(s') while copying out of PSUM
                nc.vector.tensor_scalar_add(
                    dlp_stage[:, blk * TPB:(blk + 1) * TPB], tr_ps[:], ns0_t[:]
                )

                dzv = dz[blk * BLK:(blk + 1) * BLK, :].rearrange(
                    "(p r) d -> p (r d)", p=128
                )
                # stores go on the ACT HWDGE queue so the sync queue's
                # transposes can never head-of-line block them
                nc.scalar.dma_start(dzv, dz_stage[:])

            dlv = dlogp.rearrange(
                "(k p r) o -> p k (r o)", k=N_BLOCKS, p=128, r=TPB
            )
            nc.sync.dma_start(
                dlv, dlp_stage[:].rearrange("p (k r) -> p k r", k=N_BLOCKS)
            )

    _fix_ctrl_waits(nc)
    return nc


# ---------------------------------------------------------------------------
# stage A: hypernet (jax shard_map over the 8 cores) -> derived params
# ---------------------------------------------------------------------------

def _hyper(t, fc1_w, fc1_b, fc3_w, fc3_b):
    import jax
    import jax.numpy as jnp
    from jax.sharding import Mesh, PartitionSpec as P
    try:
        from jax.experimental.shard_map import shard_map
    except ImportError:
        from jax.sharding import shard_map

    if "hyper" not in _CACHE:
        devs = jax.devices()[:N_CORES]
        mesh = Mesh(np.array(devs), ("c",))

        def f(t, fc1_w, fc1_b, fc3w_s, fc3b_s):
            a = jnp.tanh(t.reshape(1, 1) @ fc1_w.T + fc1_b)       # [1, 512]
            ps = (a @ fc3w_s.T).reshape(-1) + fc3b_s              # [2064]
            p = jax.lax.all_gather(ps, "c").reshape(-1)           # [16512]
            W = p[:BLOCKSIZE].reshape(WIDTH, IN_OUT_DIM)
            U = p[BLOCKSIZE:2 * BLOCKSIZE].reshape(WIDTH, IN_OUT_DIM)
            B = p[2 * BLOCKSIZE:].reshape(WIDTH, 1)
            s = jnp.sum(W * U, axis=1)
            wt = W.T.astype(jnp.bfloat16)                         # [64, 128]
            wt2 = jnp.concatenate([wt, wt], axis=0)               # [128, 128]
            ubv = (U / WIDTH).astype(jnp.bfloat16)                # [128, 64]
            sbv = (s / WIDTH).astype(jnp.bfloat16)                # [128]
            neg_s0 = -jnp.sum(sbv.astype(jnp.float32))
            return (
                wt2,
                B,
                ubv,
                sbv.reshape(WIDTH, 1),
                jnp.full((128, 1), neg_s0, jnp.float32),
            )

        rep = P()
        _CACHE["hyper"] = jax.jit(shard_map(
            f, mesh=mesh,
            in_specs=(rep, rep, rep, P("c"), P("c")),
            out_specs=(rep, rep, rep, rep, rep),
            check_rep=False,
        ))
    out = _CACHE["hyper"](t, fc1_w, fc1_b, fc3_w, fc3_b)
    return [np.asarray(o) for o in out]


# ---------------------------------------------------------------------------
# entry point
# ---------------------------------------------------------------------------

def _install_ntff_hook():
    """antenv.axon_hooks (the hook registry) isn't shipped in this image;
    recreate it and wire the ctypes NTFF hook so trace=True works."""
    if _CACHE.get("ntff_hook_done"):
        return
    _CACHE["ntff_hook_done"] = True
    try:
        import types
        import antenv
        if "antenv.axon_hooks" not in sys.modules:
            mod = types.ModuleType("antenv.axon_hooks")
            mod._hook = None
            def set_axon_ntff_profile_hook(h):
                mod._hook = h
            def get_axon_ntff_profile_hook():
                return mod._hook
            mod.set_axon_ntff_profile_hook = set_axon_ntff_profile_hook
            mod.get_axon_ntff_profile_hook = get_axon_ntff_profile_hook
            sys.modules["antenv.axon_hooks"] = mod
            antenv.axon_hooks = mod
        from trn_agent_boot.trn_boot import _ntff_profile_via_ctypes
        hook = _ntff_profile_via_ctypes("/opt/axon/libaxon_pjrt.so")
        sys.modules["antenv.axon_hooks"].set_axon_ntff_profile_hook(hook)
    except Exception:
        pass


def kernel(t, z, logp_z, fc1_w, fc1_b, fc3_w, fc3_b, _trace=False):
    _enable_jax_cache()
    if _trace:
        _install_ntff_hook()
    from concourse.bass_utils import run_bass_kernel_spmd

    wt2, B, ub, sbv, neg_s0 = _hyper(
        np.asarray(t, np.float32), np.asarray(fc1_w, np.float32),
        np.asarray(fc1_b, np.float32), np.asarray(fc3_w, np.float32),
        np.asarray(fc3_b, np.float32),
    )

    if "nc" not in _CACHE:
        _CACHE["nc"] = _build_bass()
    nc = _CACHE["nc"]

    z = np.asarray(z, np.float32)
    common = dict(wt2=wt2, bias=B, ub=ub, sb=sbv, neg_s0=neg_s0)
    in_maps = [
        dict(z=z[i * BC:(i + 1) * BC], **common) for i in range(N_CORES)
    ]
    res = run_bass_kernel_spmd(nc, in_maps, list(range(N_CORES)), trace=_trace)
    dz = np.concatenate([res.results[i]["dz"] for i in range(N_CORES)], axis=0)
    dlogp = np.concatenate(
        [res.results[i]["dlogp"] for i in range(N_CORES)], axis=0
    )
    if _trace:
        kernel.last_exec_time_ns = res.exec_time_ns
        kernel.last_results = res
    return dz, dlogp


# revision 14
# speedup vs baseline: 1.1505x; 1.1505x over previous
"""CNF2 (continuous normalizing flow step) Trainium2 kernel, 8-core data-parallel.

reference:
    p  = fc3(tanh(fc1(t)))                        # hypernet -> W,U,B (tiny)
    h  = tanh(W @ z.T + B)                        # [128, B]
    dz = (h.T @ U) / 128                          # [B, 64]
    dlogp = -(1/128) * (1 - h*h).T @ rowsum(W*U)  # [B, 1]

Plan:
  stage A (jax shard_map, 8 cores): hypernet with fc3 row-sharded +
    all_gather, then derived replicated params (W.T x2 bf16, B,
    U/128 bf16, s/128 bf16, -sum(s/128)).  0.2% of FLOPs.
  stage B (bass, 8 cores SPMD): batch sharded 32768 rows/core.
    Per 4096-row block: SWDGE cast-DMA loads z as bf16 [128, 2048]
    (partition p = 32 contiguous rows -> fully contiguous reads), one
    xbar DMA-transpose instruction yields all 16 z.T tiles ([128,16,128]
    batched form), h-matmul with W.T stationary (two N=512 bf16 matmuls
    per 1024-batch superstep), tanh on ACT (bias fused, bf16 out),
    h*h split DVE/GPSIMD, dz/trace matmuls with per-tile h/hh stationary
    (bf16 -> fast weight load), dz/dlogp staged in SBUF so stores are
    again fully contiguous 1MB DMAs.  The batch permutation introduced
    by the transpose tiling cancels between load and store layouts.
"""
import os
import sys
import time
from contextlib import ExitStack

for _p in ("/opt/trn_rl_repo", "/root/.axon_site/_ro/trn_rl_repo"):
    if os.path.isdir(_p) and _p not in sys.path:
        sys.path.insert(0, _p)

import numpy as np

IN_OUT_DIM = 64
HIDDEN_DIM = 512
WIDTH = 128
BATCH = 262144
BLOCKSIZE = WIDTH * IN_OUT_DIM
N_CORES = 8
BC = BATCH // N_CORES          # 32768 rows per core
BLK = 2048                     # batch rows per block
N_BLOCKS = BC // BLK           # 16
TPB = BLK // 128               # 16 transpose tiles per block
N_SS = 2                       # supersteps per block (8 tiles each)
LOOKAHEAD = 2                  # blocks of z load/transpose ahead of compute
# tile order produced by the two wide h-matmuls (even halves then odd)
SIGMA = (0, 2, 4, 6, 1, 3, 5, 7)

_CACHE = {}


def _enable_jax_cache():
    try:
        import jax
        d = "/tmp/jax_cache_cnf2"
        os.makedirs(d, exist_ok=True)
        jax.config.update("jax_compilation_cache_dir", d)
        jax.config.update("jax_persistent_cache_min_entry_size_bytes", -1)
        jax.config.update("jax_persistent_cache_min_compile_time_secs", 0)
    except Exception:
        pass


# ---------------------------------------------------------------------------
# workarounds: this walrus build accepts at most ONE sync-wait per
# instruction on CTRL templates; Tile's tail drain attaches one per live
# semaphore.  Split extra waits onto same-engine NoOps (same-engine program
# order preserves the wait-before-execute semantics).
# ---------------------------------------------------------------------------

def _install_tile_patch():
    import concourse.tile as tile
    from concourse import mybir
    from concourse.vector_clock import ScopedClock

    def _patched(self, tick_clock, wait_clock):
        drain_inst = self.nc.sync.drain()
        wait_clock.add_sem_waits(
            drain_inst.ins, ScopedClock({None: tick_clock.global_clock})
        )
        si = drain_inst.ins.sync_info
        waits = list(si.on_wait or []) if si is not None else []
        if len(waits) > 1:
            si.on_wait = waits[:1]
            for w in waits[1:]:
                d2 = self.nc.sync.drain()
                if d2.ins.sync_info is None:
                    d2.ins.sync_info = mybir.SyncInfo(on_wait=[w], on_update=[])
                else:
                    d2.ins.sync_info.on_wait = [w]
        self.nc.all_engine_barrier()
        assert self.sems is not None
        popped = self.nc._tile_sem_poison_stack.pop()
        assert popped is self._sem_poison
        self.nc.clear_and_free_semaphores(list(self.sems.allocated().values()))
        self.nc.all_engine_barrier()

    tile.TileContext._drain_and_barrier = _patched


def _fix_ctrl_waits(nc, max_waits=1):
    from concourse import mybir
    n_fixed = 0
    for fn in nc.m.functions:
        for bb in fn.blocks:
            out = []
            for inst in bb.instructions:
                si = inst.sync_info
                waits = list(si.on_wait) if (si is not None and si.on_wait) else []
                if len(waits) > max_waits:
                    keep, rest = waits[:max_waits], waits[max_waits:]
                    k = 0
                    while rest:
                        chunk, rest = rest[:max_waits], rest[max_waits:]
                        nop = mybir.InstNoOp(name=f"{inst.name}-xw{k}")
                        nop.engine = inst.engine
                        nop.sync_info = mybir.SyncInfo(on_wait=chunk, on_update=[])
                        out.append(nop)
                        k += 1
                    si.on_wait = keep
                    n_fixed += 1
                out.append(inst)
            bb.instructions = out
    return n_fixed


# ---------------------------------------------------------------------------
# stage B: the main bass kernel
# ---------------------------------------------------------------------------

def _build_bass():
    import concourse.bass as bass
    import concourse.tile as tile
    from concourse import mybir

    _install_tile_patch()

    f32 = mybir.dt.float32
    bf16 = mybir.dt.bfloat16
    nc = bass.Bass(num_devices=N_CORES)

    z = nc.declare_dram_parameter("z", [BC, IN_OUT_DIM], f32, isOutput=False)
    wt2 = nc.declare_dram_parameter("wt2", [128, 128], bf16, isOutput=False)
    bias = nc.declare_dram_parameter("bias", [128, 1], f32, isOutput=False)
    ub = nc.declare_dram_parameter("ub", [128, 64], bf16, isOutput=False)
    sb = nc.declare_dram_parameter("sb", [128, 1], bf16, isOutput=False)
    neg_s0 = nc.declare_dram_parameter("neg_s0", [128, 1], f32, isOutput=False)
    dz = nc.declare_dram_parameter("dz", [BC, IN_OUT_DIM], f32, isOutput=True)
    dlogp = nc.declare_dram_parameter("dlogp", [BC, 1], f32, isOutput=True)

    Tanh = mybir.ActivationFunctionType.Tanh

    with tile.TileContext(nc) as tc:
        with ExitStack() as ctx:
            cpool = ctx.enter_context(tc.tile_pool(name="consts", bufs=1))
            zpool = ctx.enter_context(tc.tile_pool(name="zbuf", bufs=LOOKAHEAD + 1))
            ztpool = ctx.enter_context(tc.tile_pool(name="zt", bufs=LOOKAHEAD + 1))
            h_pool = ctx.enter_context(tc.tile_pool(name="h", bufs=2))
            hh_pool = ctx.enter_context(tc.tile_pool(name="hh", bufs=2))
            dzs_pool = ctx.enter_context(tc.tile_pool(name="dzs", bufs=3))
            dlp_pool = ctx.enter_context(tc.tile_pool(name="dlp", bufs=1))
            ps_hp = ctx.enter_context(tc.tile_pool(name="ps_hp", bufs=2, space="PSUM"))
            ps_dz = ctx.enter_context(tc.tile_pool(name="ps_dz", bufs=2, space="PSUM"))
            ps_tr = ctx.enter_context(tc.tile_pool(name="ps_tr", bufs=2, space="PSUM"))

            # replicated constants
            wt2_t = cpool.tile([128, 128], bf16)
            nc.sync.dma_start(wt2_t[:], wt2[:])
            ub_t = cpool.tile([128, 64], bf16)
            nc.sync.dma_start(ub_t[:], ub[:])
            sb_t = cpool.tile([128, 1], bf16)
            nc.sync.dma_start(sb_t[:], sb[:])
            bias_t = cpool.tile([128, 1], f32)
            nc.sync.dma_start(bias_t[:], bias[:])
            ns0_t = cpool.tile([128, 1], f32)
            nc.sync.dma_start(ns0_t[:], neg_s0[:])

            dlp_stage = dlp_pool.tile([128, N_BLOCKS * TPB], f32)

            def issue_load(blk):
                """SWDGE cast-load z block -> bf16, then one xbar transpose
                instruction for all its z.T pair-tiles."""
                zv = z[blk * BLK:(blk + 1) * BLK, :].rearrange(
                    "(p r) d -> p (r d)", p=128
                )
                zbuf = zpool.tile([128, TPB * IN_OUT_DIM], bf16, tag="zbuf")
                nc.gpsimd.dma_start(zbuf[:], zv)
                zt = ztpool.tile([128, TPB * IN_OUT_DIM], bf16, tag="zt")
                nc.sync.dma_start(
                    zt[:].rearrange("p (j b) -> p j b", j=TPB // 2),
                    zbuf[:],
                    transpose=True,
                )
                return zt

            def store_dz(blk, dz_stage):
                dzv = dz[blk * BLK:(blk + 1) * BLK, :].rearrange(
                    "(p r) d -> p (r d)", p=128
                )
                nc.sync.dma_start(dzv, dz_stage[:])

            zts = {}
            for blk in range(LOOKAHEAD):
                zts[blk] = issue_load(blk)

            for blk in range(N_BLOCKS):
                if blk + LOOKAHEAD < N_BLOCKS:
                    zts[blk + LOOKAHEAD] = issue_load(blk + LOOKAHEAD)
                zt = zts.pop(blk)
                dz_stage = dzs_pool.tile([128, TPB * IN_OUT_DIM], f32, tag="dzs")
                tr_ps = ps_tr.tile([128, TPB], f32, tag="tr")

                for ss in range(N_SS):
                    t0 = ss * 8
                    # --- h matmuls: two wide N=512 (even tiles then odd)
                    hp_ps = ps_hp.tile([128, 1024], f32, tag="hp")
                    nc.tensor.matmul(
                        hp_ps[:, 0:512], wt2_t[0:64, :],
                        zt[0:64, ss * 512:(ss + 1) * 512],
                    )
                    nc.tensor.matmul(
                        hp_ps[:, 512:1024], wt2_t[64:128, :],
                        zt[64:128, ss * 512:(ss + 1) * 512],
                    )

                    # --- tanh (+bias) -> bf16 h
                    h_sb = h_pool.tile([128, 1024], bf16, tag="h")
                    nc.scalar.activation(h_sb[:], hp_ps[:], Tanh, bias=bias_t[:])

                    # --- hh = h*h split between gpsimd and DVE
                    hh_sb = hh_pool.tile([128, 1024], bf16, tag="hh")
                    nc.gpsimd.tensor_mul(
                        hh_sb[:, 0:512], h_sb[:, 0:512], h_sb[:, 0:512]
                    )
                    nc.vector.tensor_mul(
                        hh_sb[:, 512:1024], h_sb[:, 512:1024], h_sb[:, 512:1024]
                    )

                    # --- dz + trace matmuls per 128-batch tile
                    dz_ps = ps_dz.tile([128, 512], f32, tag="dzp")
                    for k in range(8):
                        lh = h_sb[:, k * 128:(k + 1) * 128]
                        nc.tensor.matmul(
                            dz_ps[:, SIGMA[k] * 64:SIGMA[k] * 64 + 64], lh, ub_t[:]
                        )
                    for k in range(8):
                        t = t0 + SIGMA[k]
                        lhh = hh_sb[:, k * 128:(k + 1) * 128]
                        nc.tensor.matmul(tr_ps[:, t:t + 1], lhh, sb_t[:])

                    nc.vector.tensor_copy(
                        dz_stage[:, ss * 512:(ss + 1) * 512], dz_ps[:]
                    )

                # dlogp for the block: add -sum(s') while copying out of PSUM
                nc.vector.tensor_scalar_add(
                    dlp_stage[:, blk * TPB:(blk + 1) * TPB], tr_ps[:], ns0_t[:]
                )

                store_dz(blk, dz_stage)

            dlv = dlogp.rearrange(
                "(k p r) o -> p k (r o)", k=N_BLOCKS, p=128, r=TPB
            )
            nc.sync.dma_start(
                dlv, dlp_stage[:].rearrange("p (k r) -> p k r", k=N_BLOCKS)
            )

    _fix_ctrl_waits(nc)
    return nc


# ---------------------------------------------------------------------------
# stage A: hypernet (jax shard_map over the 8 cores) -> derived params
# ---------------------------------------------------------------------------

def _hyper(t, fc1_w, fc1_b, fc3_w, fc3_b):
    import jax
    import jax.numpy as jnp
    from jax.sharding import Mesh, PartitionSpec as P
    try:
        from jax.experimental.shard_map import shard_map
    except ImportError:
        from jax.sharding import shard_map

    if "hyper" not in _CACHE:
        devs = jax.devices()[:N_CORES]
        mesh = Mesh(np.array(devs), ("c",))

        def f(t, fc1_w, fc1_b, fc3w_s, fc3b_s):
            a = jnp.tanh(t.reshape(1, 1) @ fc1_w.T + fc1_b)       # [1, 512]
            ps = (a @ fc3w_s.T).reshape(-1) + fc3b_s              # [2064]
            p = jax.lax.all_gather(ps, "c").reshape(-1)           # [16512]
            W = p[:BLOCKSIZE].reshape(WIDTH, IN_OUT_DIM)
            U = p[BLOCKSIZE:2 * BLOCKSIZE].reshape(WIDTH, IN_OUT_DIM)
            B = p[2 * BLOCKSIZE:].reshape(WIDTH, 1)
            s = jnp.sum(W * U, axis=1)
            wt = W.T.astype(jnp.bfloat16)                         # [64, 128]
            wt2 = jnp.concatenate([wt, wt], axis=0)               # [128, 128]
            ubv = (U / WIDTH).astype(jnp.bfloat16)                # [128, 64]
            sbv = (s / WIDTH).astype(jnp.bfloat16)                # [128]
            neg_s0 = -jnp.sum(sbv.astype(jnp.float32))
            return (
                wt2,
                B,
                ubv,
                sbv.reshape(WIDTH, 1),
                jnp.full((128, 1), neg_s0, jnp.float32),
            )

        rep = P()
        _CACHE["hyper"] = jax.jit(shard_map(
            f, mesh=mesh,
            in_specs=(rep, rep, rep, P("c"), P("c")),
            out_specs=(rep, rep, rep, rep, rep),
            check_rep=False,
        ))
    out = _CACHE["hyper"](t, fc1_w, fc1_b, fc3_w, fc3_b)
    return [np.asarray(o) for o in out]


# ---------------------------------------------------------------------------
# entry point
# ---------------------------------------------------------------------------

def _install_ntff_hook():
    """antenv.axon_hooks (the hook registry) isn't shipped in this image;
    recreate it and wire the ctypes NTFF hook so trace=True works."""
    if _CACHE.get("ntff_hook_done"):
        return
    _CACHE["ntff_hook_done"] = True
    try:
        import types
        import antenv
        if "antenv.axon_hooks" not in sys.modules:
            mod = types.ModuleType("antenv.axon_hooks")
            mod._hook = None
            def set_axon_ntff_profile_hook(h):
                mod._hook = h
            def get_axon_ntff_profile_hook():
                return mod._hook
            mod.set_axon_ntff_profile_hook = set_axon_ntff_profile_hook
            mod.get_axon_ntff_profile_hook = get_axon_ntff_profile_hook
            sys.modules["antenv.axon_hooks"] = mod
            antenv.axon_hooks = mod
        from trn_agent_boot.trn_boot import _ntff_profile_via_ctypes
        hook = _ntff_profile_via_ctypes("/opt/axon/libaxon_pjrt.so")
        sys.modules["antenv.axon_hooks"].set_axon_ntff_profile_hook(hook)
    except Exception:
        pass


def kernel(t, z, logp_z, fc1_w, fc1_b, fc3_w, fc3_b, _trace=False):
    _enable_jax_cache()
    if _trace:
        _install_ntff_hook()
    from concourse.bass_utils import run_bass_kernel_spmd

    wt2, B, ub, sbv, neg_s0 = _hyper(
        np.asarray(t, np.float32), np.asarray(fc1_w, np.float32),
        np.asarray(fc1_b, np.float32), np.asarray(fc3_w, np.float32),
        np.asarray(fc3_b, np.float32),
    )

    if "nc" not in _CACHE:
        _CACHE["nc"] = _build_bass()
    nc = _CACHE["nc"]

    z = np.asarray(z, np.float32)
    common = dict(wt2=wt2, bias=B, ub=ub, sb=sbv, neg_s0=neg_s0)
    in_maps = [
        dict(z=z[i * BC:(i + 1) * BC], **common) for i in range(N_CORES)
    ]
    res = run_bass_kernel_spmd(nc, in_maps, list(range(N_CORES)), trace=_trace)
    dz = np.concatenate([res.results[i]["dz"] for i in range(N_CORES)], axis=0)
    dlogp = np.concatenate(
        [res.results[i]["dlogp"] for i in range(N_CORES)], axis=0
    )
    if _trace:
        kernel.last_exec_time_ns = res.exec_time_ns
        kernel.last_results = res
    return dz, dlogp


# revision 15
# speedup vs baseline: 1.3280x; 1.1543x over previous
"""CNF2 (continuous normalizing flow step) Trainium2 kernel, 8-core data-parallel.

reference:
    p  = fc3(tanh(fc1(t)))                        # hypernet -> W,U,B (tiny)
    h  = tanh(W @ z.T + B)                        # [128, B]
    dz = (h.T @ U) / 128                          # [B, 64]
    dlogp = -(1/128) * (1 - h*h).T @ rowsum(W*U)  # [B, 1]

Plan:
  stage A (jax shard_map, 8 cores): hypernet with fc3 row-sharded +
    all_gather, then derived replicated params (W.T x2 bf16, B,
    U/128 bf16, s/128 bf16, -sum(s/128)).  0.2% of FLOPs.
  stage B (bass, 8 cores SPMD): batch sharded 32768 rows/core.
    Per 4096-row block: SWDGE cast-DMA loads z as bf16 [128, 2048]
    (partition p = 32 contiguous rows -> fully contiguous reads), one
    xbar DMA-transpose instruction yields all 16 z.T tiles ([128,16,128]
    batched form), h-matmul with W.T stationary (two N=512 bf16 matmuls
    per 1024-batch superstep), tanh on ACT (bias fused, bf16 out),
    h*h split DVE/GPSIMD, dz/trace matmuls with per-tile h/hh stationary
    (bf16 -> fast weight load), dz/dlogp staged in SBUF so stores are
    again fully contiguous 1MB DMAs.  The batch permutation introduced
    by the transpose tiling cancels between load and store layouts.
"""
import os
import sys
import time
from contextlib import ExitStack

for _p in ("/opt/trn_rl_repo", "/root/.axon_site/_ro/trn_rl_repo"):
    if os.path.isdir(_p) and _p not in sys.path:
        sys.path.insert(0, _p)

import numpy as np

IN_OUT_DIM = 64
HIDDEN_DIM = 512
WIDTH = 128
BATCH = 262144
BLOCKSIZE = WIDTH * IN_OUT_DIM
N_CORES = 8
BC = BATCH // N_CORES          # 32768 rows per core
BLK = 4096                     # batch rows per block
N_BLOCKS = BC // BLK
TPB = BLK // 128
N_SS = BLK // 1024             # supersteps per block (8 tiles each)
LOOKAHEAD = 1                  # blocks of z load/transpose ahead of compute
# tile order produced by the two wide h-matmuls (even halves then odd)
SIGMA = (0, 2, 4, 6, 1, 3, 5, 7)

_CACHE = {}


def _enable_jax_cache():
    try:
        import jax
        d = "/tmp/jax_cache_cnf2"
        os.makedirs(d, exist_ok=True)
        jax.config.update("jax_compilation_cache_dir", d)
        jax.config.update("jax_persistent_cache_min_entry_size_bytes", -1)
        jax.config.update("jax_persistent_cache_min_compile_time_secs", 0)
    except Exception:
        pass


# ---------------------------------------------------------------------------
# workarounds: this walrus build accepts at most ONE sync-wait per
# instruction on CTRL templates; Tile's tail drain attaches one per live
# semaphore.  Split extra waits onto same-engine NoOps (same-engine program
# order preserves the wait-before-execute semantics).
# ---------------------------------------------------------------------------

def _install_tile_patch():
    import concourse.tile as tile
    from concourse import mybir
    from concourse.vector_clock import ScopedClock

    def _patched(self, tick_clock, wait_clock):
        drain_inst = self.nc.sync.drain()
        wait_clock.add_sem_waits(
            drain_inst.ins, ScopedClock({None: tick_clock.global_clock})
        )
        si = drain_inst.ins.sync_info
        waits = list(si.on_wait or []) if si is not None else []
        if len(waits) > 1:
            si.on_wait = waits[:1]
            for w in waits[1:]:
                d2 = self.nc.sync.drain()
                if d2.ins.sync_info is None:
                    d2.ins.sync_info = mybir.SyncInfo(on_wait=[w], on_update=[])
                else:
                    d2.ins.sync_info.on_wait = [w]
        self.nc.all_engine_barrier()
        assert self.sems is not None
        popped = self.nc._tile_sem_poison_stack.pop()
        assert popped is self._sem_poison
        self.nc.clear_and_free_semaphores(list(self.sems.allocated().values()))
        self.nc.all_engine_barrier()

    tile.TileContext._drain_and_barrier = _patched


def _fix_ctrl_waits(nc, max_waits=1):
    from concourse import mybir
    n_fixed = 0
    for fn in nc.m.functions:
        for bb in fn.blocks:
            out = []
            for inst in bb.instructions:
                si = inst.sync_info
                waits = list(si.on_wait) if (si is not None and si.on_wait) else []
                if len(waits) > max_waits:
                    keep, rest = waits[:max_waits], waits[max_waits:]
                    k = 0
                    while rest:
                        chunk, rest = rest[:max_waits], rest[max_waits:]
                        nop = mybir.InstNoOp(name=f"{inst.name}-xw{k}")
                        nop.engine = inst.engine
                        nop.sync_info = mybir.SyncInfo(on_wait=chunk, on_update=[])
                        out.append(nop)
                        k += 1
                    si.on_wait = keep
                    n_fixed += 1
                out.append(inst)
            bb.instructions = out
    return n_fixed


# ---------------------------------------------------------------------------
# stage B: the main bass kernel
# ---------------------------------------------------------------------------

def _build_bass():
    import concourse.bass as bass
    import concourse.tile as tile
    from concourse import mybir

    _install_tile_patch()

    f32 = mybir.dt.float32
    bf16 = mybir.dt.bfloat16
    nc = bass.Bass(num_devices=N_CORES)

    z = nc.declare_dram_parameter("z", [BC, IN_OUT_DIM], f32, isOutput=False)
    wt2 = nc.declare_dram_parameter("wt2", [128, 128], bf16, isOutput=False)
    bias = nc.declare_dram_parameter("bias", [128, 1], f32, isOutput=False)
    ub = nc.declare_dram_parameter("ub", [128, 64], bf16, isOutput=False)
    sb = nc.declare_dram_parameter("sb", [128, 1], bf16, isOutput=False)
    neg_s0 = nc.declare_dram_parameter("neg_s0", [128, 1], f32, isOutput=False)
    dz = nc.declare_dram_parameter("dz", [BC, IN_OUT_DIM], f32, isOutput=True)
    dlogp = nc.declare_dram_parameter("dlogp", [BC, 1], f32, isOutput=True)

    Tanh = mybir.ActivationFunctionType.Tanh

    with tile.TileContext(nc) as tc:
        with ExitStack() as ctx:
            cpool = ctx.enter_context(tc.tile_pool(name="consts", bufs=1))
            zpool = ctx.enter_context(tc.tile_pool(name="zbuf", bufs=LOOKAHEAD + 1))
            ztpool = ctx.enter_context(tc.tile_pool(name="zt", bufs=LOOKAHEAD + 1))
            h_pool = ctx.enter_context(tc.tile_pool(name="h", bufs=2))
            hh_pool = ctx.enter_context(tc.tile_pool(name="hh", bufs=2))
            dzs_pool = ctx.enter_context(tc.tile_pool(name="dzs", bufs=3))
            dlp_pool = ctx.enter_context(tc.tile_pool(name="dlp", bufs=1))
            ps_hp = ctx.enter_context(tc.tile_pool(name="ps_hp", bufs=2, space="PSUM"))
            ps_dz = ctx.enter_context(tc.tile_pool(name="ps_dz", bufs=2, space="PSUM"))
            ps_tr = ctx.enter_context(tc.tile_pool(name="ps_tr", bufs=2, space="PSUM"))

            # replicated constants
            wt2_t = cpool.tile([128, 128], bf16)
            nc.sync.dma_start(wt2_t[:], wt2[:])
            ub_t = cpool.tile([128, 64], bf16)
            nc.sync.dma_start(ub_t[:], ub[:])
            sb_t = cpool.tile([128, 1], bf16)
            nc.sync.dma_start(sb_t[:], sb[:])
            bias_t = cpool.tile([128, 1], f32)
            nc.sync.dma_start(bias_t[:], bias[:])
            ns0_t = cpool.tile([128, 1], f32)
            nc.sync.dma_start(ns0_t[:], neg_s0[:])

            dlp_stage = dlp_pool.tile([128, N_BLOCKS * TPB], f32)

            def issue_load(blk):
                """SWDGE cast-load z block -> bf16, then one xbar transpose
                instruction for all its z.T pair-tiles."""
                zv = z[blk * BLK:(blk + 1) * BLK, :].rearrange(
                    "(p r) d -> p (r d)", p=128
                )
                zbuf = zpool.tile([128, TPB * IN_OUT_DIM], bf16, tag="zbuf")
                nc.gpsimd.dma_start(zbuf[:], zv)
                zt = ztpool.tile([128, TPB * IN_OUT_DIM], bf16, tag="zt")
                nc.sync.dma_start(
                    zt[:].rearrange("p (j b) -> p j b", j=TPB // 2),
                    zbuf[:],
                    transpose=True,
                )
                return zt

            def store_dz(blk, dz_stage):
                dzv = dz[blk * BLK:(blk + 1) * BLK, :].rearrange(
                    "(p r) d -> p (r d)", p=128
                )
                nc.sync.dma_start(dzv, dz_stage[:])

            zts = {}
            for blk in range(LOOKAHEAD):
                zts[blk] = issue_load(blk)

            for blk in range(N_BLOCKS):
                if blk + LOOKAHEAD < N_BLOCKS:
                    zts[blk + LOOKAHEAD] = issue_load(blk + LOOKAHEAD)
                zt = zts.pop(blk)
                dz_stage = dzs_pool.tile([128, TPB * IN_OUT_DIM], f32, tag="dzs")
                tr_ps = ps_tr.tile([128, TPB], f32, tag="tr")

                for ss in range(N_SS):
                    t0 = ss * 8
                    # --- h matmuls: two wide N=512 (even tiles then odd)
                    hp_ps = ps_hp.tile([128, 1024], f32, tag="hp")
                    nc.tensor.matmul(
                        hp_ps[:, 0:512], wt2_t[0:64, :],
                        zt[0:64, ss * 512:(ss + 1) * 512],
                    )
                    nc.tensor.matmul(
                        hp_ps[:, 512:1024], wt2_t[64:128, :],
                        zt[64:128, ss * 512:(ss + 1) * 512],
                    )

                    # --- tanh (+bias) -> bf16 h
                    h_sb = h_pool.tile([128, 1024], bf16, tag="h")
                    nc.scalar.activation(h_sb[:], hp_ps[:], Tanh, bias=bias_t[:])

                    # --- hh = h*h split between gpsimd and DVE
                    hh_sb = hh_pool.tile([128, 1024], bf16, tag="hh")
                    nc.gpsimd.tensor_mul(
                        hh_sb[:, 0:512], h_sb[:, 0:512], h_sb[:, 0:512]
                    )
                    nc.vector.tensor_mul(
                        hh_sb[:, 512:1024], h_sb[:, 512:1024], h_sb[:, 512:1024]
                    )

                    # --- dz + trace matmuls per 128-batch tile
                    dz_ps = ps_dz.tile([128, 512], f32, tag="dzp")
                    for k in range(8):
                        lh = h_sb[:, k * 128:(k + 1) * 128]
                        nc.tensor.matmul(
                            dz_ps[:, SIGMA[k] * 64:SIGMA[k] * 64 + 64], lh, ub_t[:]
                        )
                    for k in range(8):
                        t = t0 + SIGMA[k]
                        lhh = hh_sb[:, k * 128:(k + 1) * 128]
                        nc.tensor.matmul(tr_ps[:, t:t + 1], lhh, sb_t[:])

                    nc.vector.tensor_copy(
                        dz_stage[:, ss * 512:(ss + 1) * 512], dz_ps[:]
                    )

                # dlogp for the block: add -sum(s') while copying out of PSUM
                nc.vector.tensor_scalar_add(
                    dlp_stage[:, blk * TPB:(blk + 1) * TPB], tr_ps[:], ns0_t[:]
                )

                store_dz(blk, dz_stage)

            dlv = dlogp.rearrange(
                "(k p r) o -> p k (r o)", k=N_BLOCKS, p=128, r=TPB
            )
            nc.sync.dma_start(
                dlv, dlp_stage[:].rearrange("p (k r) -> p k r", k=N_BLOCKS)
            )

    _fix_ctrl_waits(nc)
    return nc


# ---------------------------------------------------------------------------
# stage A: hypernet (jax shard_map over the 8 cores) -> derived params
# ---------------------------------------------------------------------------

def _hyper(t, fc1_w, fc1_b, fc3_w, fc3_b):
    import jax
    import jax.numpy as jnp
    from jax.sharding import Mesh, PartitionSpec as P
    try:
        from jax.experimental.shard_map import shard_map
    except ImportError:
        from jax.sharding import shard_map

    if "hyper" not in _CACHE:
        devs = jax.devices()[:N_CORES]
        mesh = Mesh(np.array(devs), ("c",))

        def f(t, fc1_w, fc1_b, fc3w_s, fc3b_s):
            a = jnp.tanh(t.reshape(1, 1) @ fc1_w.T + fc1_b)       # [1, 512]
            ps = (a @ fc3w_s.T).reshape(-1) + fc3b_s              # [2064]
            p = jax.lax.all_gather(ps, "c").reshape(-1)           # [16512]
            W = p[:BLOCKSIZE].reshape(WIDTH, IN_OUT_DIM)
            U = p[BLOCKSIZE:2 * BLOCKSIZE].reshape(WIDTH, IN_OUT_DIM)
            B = p[2 * BLOCKSIZE:].reshape(WIDTH, 1)
            s = jnp.sum(W * U, axis=1)
            wt = W.T.astype(jnp.bfloat16)                         # [64, 128]
            wt2 = jnp.concatenate([wt, wt], axis=0)               # [128, 128]
            ubv = (U / WIDTH).astype(jnp.bfloat16)                # [128, 64]
            sbv = (s / WIDTH).astype(jnp.bfloat16)                # [128]
            neg_s0 = -jnp.sum(sbv.astype(jnp.float32))
            return (
                wt2,
                B,
                ubv,
                sbv.reshape(WIDTH, 1),
                jnp.full((128, 1), neg_s0, jnp.float32),
            )

        rep = P()
        _CACHE["hyper"] = jax.jit(shard_map(
            f, mesh=mesh,
            in_specs=(rep, rep, rep, P("c"), P("c")),
            out_specs=(rep, rep, rep, rep, rep),
            check_rep=False,
        ))
    out = _CACHE["hyper"](t, fc1_w, fc1_b, fc3_w, fc3_b)
    return [np.asarray(o) for o in out]


# ---------------------------------------------------------------------------
# entry point
# ---------------------------------------------------------------------------

def _install_ntff_hook():
    """antenv.axon_hooks (the hook registry) isn't shipped in this image;
    recreate it and wire the ctypes NTFF hook so trace=True works."""
    if _CACHE.get("ntff_hook_done"):
        return
    _CACHE["ntff_hook_done"] = True
    try:
        import types
        import antenv
        if "antenv.axon_hooks" not in sys.modules:
            mod = types.ModuleType("antenv.axon_hooks")
            mod._hook = None
            def set_axon_ntff_profile_hook(h):
                mod._hook = h
            def get_axon_ntff_profile_hook():
                return mod._hook
            mod.set_axon_ntff_profile_hook = set_axon_ntff_profile_hook
            mod.get_axon_ntff_profile_hook = get_axon_ntff_profile_hook
            sys.modules["antenv.axon_hooks"] = mod
            antenv.axon_hooks = mod
        from trn_agent_boot.trn_boot import _ntff_profile_via_ctypes
        hook = _ntff_profile_via_ctypes("/opt/axon/libaxon_pjrt.so")
        sys.modules["antenv.axon_hooks"].set_axon_ntff_profile_hook(hook)
    except Exception:
        pass


def kernel(t, z, logp_z, fc1_w, fc1_b, fc3_w, fc3_b, _trace=False):
    _enable_jax_cache()
    if _trace:
        _install_ntff_hook()
    from concourse.bass_utils import run_bass_kernel_spmd

    wt2, B, ub, sbv, neg_s0 = _hyper(
        np.asarray(t, np.float32), np.asarray(fc1_w, np.float32),
        np.asarray(fc1_b, np.float32), np.asarray(fc3_w, np.float32),
        np.asarray(fc3_b, np.float32),
    )

    if "nc" not in _CACHE:
        _CACHE["nc"] = _build_bass()
    nc = _CACHE["nc"]

    z = np.asarray(z, np.float32)
    common = dict(wt2=wt2, bias=B, ub=ub, sb=sbv, neg_s0=neg_s0)
    in_maps = [
        dict(z=z[i * BC:(i + 1) * BC], **common) for i in range(N_CORES)
    ]
    res = run_bass_kernel_spmd(nc, in_maps, list(range(N_CORES)), trace=_trace)
    dz = np.concatenate([res.results[i]["dz"] for i in range(N_CORES)], axis=0)
    dlogp = np.concatenate(
        [res.results[i]["dlogp"] for i in range(N_CORES)], axis=0
    )
    if _trace:
        kernel.last_exec_time_ns = res.exec_time_ns
        kernel.last_results = res
    return dz, dlogp


# revision 16
# speedup vs baseline: 1.3970x; 1.0519x over previous
"""CNF2 (continuous normalizing flow step) Trainium2 kernel, 8-core data-parallel.

reference:
    p  = fc3(tanh(fc1(t)))                        # hypernet -> W,U,B (tiny)
    h  = tanh(W @ z.T + B)                        # [128, B]
    dz = (h.T @ U) / 128                          # [B, 64]
    dlogp = -(1/128) * (1 - h*h).T @ rowsum(W*U)  # [B, 1]

Plan:
  stage A (jax shard_map, 8 cores): hypernet with fc3 row-sharded +
    all_gather, then derived replicated params (W.T x2 bf16, B,
    U/128 bf16, s/128 bf16, -sum(s/128)).  0.2% of FLOPs.
  stage B (bass, 8 cores SPMD): batch sharded 32768 rows/core.
    Per 4096-row block: SWDGE cast-DMA loads z as bf16 [128, 2048]
    (partition p = 32 contiguous rows -> fully contiguous reads), one
    xbar DMA-transpose instruction yields all 16 z.T tiles ([128,16,128]
    batched form), h-matmul with W.T stationary (two N=512 bf16 matmuls
    per 1024-batch superstep), tanh on ACT (bias fused, bf16 out),
    h*h split DVE/GPSIMD, dz/trace matmuls with per-tile h/hh stationary
    (bf16 -> fast weight load), dz/dlogp staged in SBUF so stores are
    again fully contiguous 1MB DMAs.  The batch permutation introduced
    by the transpose tiling cancels between load and store layouts.
"""
import os
import sys
import time
from contextlib import ExitStack

for _p in ("/opt/trn_rl_repo", "/root/.axon_site/_ro/trn_rl_repo"):
    if os.path.isdir(_p) and _p not in sys.path:
        sys.path.insert(0, _p)

import numpy as np

IN_OUT_DIM = 64
HIDDEN_DIM = 512
WIDTH = 128
BATCH = 262144
BLOCKSIZE = WIDTH * IN_OUT_DIM
N_CORES = 8
BC = BATCH // N_CORES          # 32768 rows per core
BLK = 4096                     # batch rows per block
N_BLOCKS = BC // BLK
TPB = BLK // 128
N_SS = BLK // 1024             # supersteps per block (8 tiles each)
LOOKAHEAD = 2                  # blocks of z load/transpose ahead of compute
# tile order produced by the two wide h-matmuls (even halves then odd)
SIGMA = (0, 2, 4, 6, 1, 3, 5, 7)

_CACHE = {}


def _enable_jax_cache():
    try:
        import jax
        d = "/tmp/jax_cache_cnf2"
        os.makedirs(d, exist_ok=True)
        jax.config.update("jax_compilation_cache_dir", d)
        jax.config.update("jax_persistent_cache_min_entry_size_bytes", -1)
        jax.config.update("jax_persistent_cache_min_compile_time_secs", 0)
    except Exception:
        pass


# ---------------------------------------------------------------------------
# workarounds: this walrus build accepts at most ONE sync-wait per
# instruction on CTRL templates; Tile's tail drain attaches one per live
# semaphore.  Split extra waits onto same-engine NoOps (same-engine program
# order preserves the wait-before-execute semantics).
# ---------------------------------------------------------------------------

def _install_tile_patch():
    import concourse.tile as tile
    from concourse import mybir
    from concourse.vector_clock import ScopedClock

    def _patched(self, tick_clock, wait_clock):
        drain_inst = self.nc.sync.drain()
        wait_clock.add_sem_waits(
            drain_inst.ins, ScopedClock({None: tick_clock.global_clock})
        )
        si = drain_inst.ins.sync_info
        waits = list(si.on_wait or []) if si is not None else []
        if len(waits) > 1:
            si.on_wait = waits[:1]
            for w in waits[1:]:
                d2 = self.nc.sync.drain()
                if d2.ins.sync_info is None:
                    d2.ins.sync_info = mybir.SyncInfo(on_wait=[w], on_update=[])
                else:
                    d2.ins.sync_info.on_wait = [w]
        self.nc.all_engine_barrier()
        assert self.sems is not None
        popped = self.nc._tile_sem_poison_stack.pop()
        assert popped is self._sem_poison
        self.nc.clear_and_free_semaphores(list(self.sems.allocated().values()))
        self.nc.all_engine_barrier()

    tile.TileContext._drain_and_barrier = _patched


def _fix_ctrl_waits(nc, max_waits=1):
    from concourse import mybir
    n_fixed = 0
    for fn in nc.m.functions:
        for bb in fn.blocks:
            out = []
            for inst in bb.instructions:
                si = inst.sync_info
                waits = list(si.on_wait) if (si is not None and si.on_wait) else []
                if len(waits) > max_waits:
                    keep, rest = waits[:max_waits], waits[max_waits:]
                    k = 0
                    while rest:
                        chunk, rest = rest[:max_waits], rest[max_waits:]
                        nop = mybir.InstNoOp(name=f"{inst.name}-xw{k}")
                        nop.engine = inst.engine
                        nop.sync_info = mybir.SyncInfo(on_wait=chunk, on_update=[])
                        out.append(nop)
                        k += 1
                    si.on_wait = keep
                    n_fixed += 1
                out.append(inst)
            bb.instructions = out
    return n_fixed


# ---------------------------------------------------------------------------
# stage B: the main bass kernel
# ---------------------------------------------------------------------------

def _build_bass():
    import concourse.bass as bass
    import concourse.tile as tile
    from concourse import mybir

    _install_tile_patch()

    f32 = mybir.dt.float32
    bf16 = mybir.dt.bfloat16
    nc = bass.Bass(num_devices=N_CORES)

    z = nc.declare_dram_parameter("z", [BC, IN_OUT_DIM], f32, isOutput=False)
    wt2 = nc.declare_dram_parameter("wt2", [128, 128], bf16, isOutput=False)
    bias = nc.declare_dram_parameter("bias", [128, 1], f32, isOutput=False)
    ub = nc.declare_dram_parameter("ub", [128, 64], bf16, isOutput=False)
    sb = nc.declare_dram_parameter("sb", [128, 1], bf16, isOutput=False)
    neg_s0 = nc.declare_dram_parameter("neg_s0", [128, 1], f32, isOutput=False)
    dz = nc.declare_dram_parameter("dz", [BC, IN_OUT_DIM], f32, isOutput=True)
    dlogp = nc.declare_dram_parameter("dlogp", [BC, 1], f32, isOutput=True)

    Tanh = mybir.ActivationFunctionType.Tanh

    with tile.TileContext(nc) as tc:
        with ExitStack() as ctx:
            cpool = ctx.enter_context(tc.tile_pool(name="consts", bufs=1))
            zpool = ctx.enter_context(tc.tile_pool(name="zbuf", bufs=LOOKAHEAD + 1))
            ztpool = ctx.enter_context(tc.tile_pool(name="zt", bufs=LOOKAHEAD + 1))
            h_pool = ctx.enter_context(tc.tile_pool(name="h", bufs=2))
            hh_pool = ctx.enter_context(tc.tile_pool(name="hh", bufs=2))
            dzs_pool = ctx.enter_context(tc.tile_pool(name="dzs", bufs=3))
            dlp_pool = ctx.enter_context(tc.tile_pool(name="dlp", bufs=1))
            ps_hp = ctx.enter_context(tc.tile_pool(name="ps_hp", bufs=2, space="PSUM"))
            ps_dz = ctx.enter_context(tc.tile_pool(name="ps_dz", bufs=2, space="PSUM"))
            ps_tr = ctx.enter_context(tc.tile_pool(name="ps_tr", bufs=2, space="PSUM"))

            # replicated constants
            wt2_t = cpool.tile([128, 128], bf16)
            nc.sync.dma_start(wt2_t[:], wt2[:])
            ub_t = cpool.tile([128, 64], bf16)
            nc.sync.dma_start(ub_t[:], ub[:])
            sb_t = cpool.tile([128, 1], bf16)
            nc.sync.dma_start(sb_t[:], sb[:])
            bias_t = cpool.tile([128, 1], f32)
            nc.sync.dma_start(bias_t[:], bias[:])
            ns0_t = cpool.tile([128, 1], f32)
            nc.sync.dma_start(ns0_t[:], neg_s0[:])

            dlp_stage = dlp_pool.tile([128, N_BLOCKS * TPB], f32)

            def issue_load(blk):
                """SWDGE cast-load z block -> bf16, then one xbar transpose
                instruction for all its z.T pair-tiles."""
                zv = z[blk * BLK:(blk + 1) * BLK, :].rearrange(
                    "(p r) d -> p (r d)", p=128
                )
                zbuf = zpool.tile([128, TPB * IN_OUT_DIM], bf16, tag="zbuf")
                nc.gpsimd.dma_start(zbuf[:], zv)
                zt = ztpool.tile([128, TPB * IN_OUT_DIM], bf16, tag="zt")
                nc.sync.dma_start(
                    zt[:].rearrange("p (j b) -> p j b", j=TPB // 2),
                    zbuf[:],
                    transpose=True,
                )
                return zt

            def store_dz(blk, dz_stage):
                dzv = dz[blk * BLK:(blk + 1) * BLK, :].rearrange(
                    "(p r) d -> p (r d)", p=128
                )
                nc.sync.dma_start(dzv, dz_stage[:])

            zts = {}
            for blk in range(LOOKAHEAD):
                zts[blk] = issue_load(blk)

            for blk in range(N_BLOCKS):
                if blk + LOOKAHEAD < N_BLOCKS:
                    zts[blk + LOOKAHEAD] = issue_load(blk + LOOKAHEAD)
                zt = zts.pop(blk)
                dz_stage = dzs_pool.tile([128, TPB * IN_OUT_DIM], f32, tag="dzs")
                tr_ps = ps_tr.tile([128, TPB], f32, tag="tr")

                for ss in range(N_SS):
                    t0 = ss * 8
                    # --- h matmuls: two wide N=512 (even tiles then odd)
                    hp_ps = ps_hp.tile([128, 1024], f32, tag="hp")
                    nc.tensor.matmul(
                        hp_ps[:, 0:512], wt2_t[0:64, :],
                        zt[0:64, ss * 512:(ss + 1) * 512],
                    )
                    nc.tensor.matmul(
                        hp_ps[:, 512:1024], wt2_t[64:128, :],
                        zt[64:128, ss * 512:(ss + 1) * 512],
                    )

                    # --- tanh (+bias) -> bf16 h
                    h_sb = h_pool.tile([128, 1024], bf16, tag="h")
                    nc.scalar.activation(h_sb[:], hp_ps[:], Tanh, bias=bias_t[:])

                    # --- hh = h*h split between gpsimd and DVE
                    hh_sb = hh_pool.tile([128, 1024], bf16, tag="hh")
                    nc.gpsimd.tensor_mul(
                        hh_sb[:, 0:512], h_sb[:, 0:512], h_sb[:, 0:512]
                    )
                    nc.vector.tensor_mul(
                        hh_sb[:, 512:1024], h_sb[:, 512:1024], h_sb[:, 512:1024]
                    )

                    # --- dz + trace matmuls per 128-batch tile
                    dz_ps = ps_dz.tile([128, 512], f32, tag="dzp")
                    for k in range(8):
                        lh = h_sb[:, k * 128:(k + 1) * 128]
                        nc.tensor.matmul(
                            dz_ps[:, SIGMA[k] * 64:SIGMA[k] * 64 + 64], lh, ub_t[:]
                        )
                    for k in range(8):
                        t = t0 + SIGMA[k]
                        lhh = hh_sb[:, k * 128:(k + 1) * 128]
                        nc.tensor.matmul(tr_ps[:, t:t + 1], lhh, sb_t[:])

                    nc.vector.tensor_copy(
                        dz_stage[:, ss * 512:(ss + 1) * 512], dz_ps[:]
                    )

                # dlogp for the block: add -sum(s') while copying out of PSUM
                nc.vector.tensor_scalar_add(
                    dlp_stage[:, blk * TPB:(blk + 1) * TPB], tr_ps[:], ns0_t[:]
                )

                store_dz(blk, dz_stage)

            dlv = dlogp.rearrange(
                "(k p r) o -> p k (r o)", k=N_BLOCKS, p=128, r=TPB
            )
            nc.sync.dma_start(
                dlv, dlp_stage[:].rearrange("p (k r) -> p k r", k=N_BLOCKS)
            )

    _fix_ctrl_waits(nc)
    return nc


# ---------------------------------------------------------------------------
# stage A: hypernet (jax shard_map over the 8 cores) -> derived params
# ---------------------------------------------------------------------------

def _hyper(t, fc1_w, fc1_b, fc3_w, fc3_b):
    import jax
    import jax.numpy as jnp
    from jax.sharding import Mesh, PartitionSpec as P
    try:
        from jax.experimental.shard_map import shard_map
    except ImportError:
        from jax.sharding import shard_map

    if "hyper" not in _CACHE:
        devs = jax.devices()[:N_CORES]
        mesh = Mesh(np.array(devs), ("c",))

        def f(t, fc1_w, fc1_b, fc3w_s, fc3b_s):
            a = jnp.tanh(t.reshape(1, 1) @ fc1_w.T + fc1_b)       # [1, 512]
            ps = (a @ fc3w_s.T).reshape(-1) + fc3b_s              # [2064]
            p = jax.lax.all_gather(ps, "c").reshape(-1)           # [16512]
            W = p[:BLOCKSIZE].reshape(WIDTH, IN_OUT_DIM)
            U = p[BLOCKSIZE:2 * BLOCKSIZE].reshape(WIDTH, IN_OUT_DIM)
            B = p[2 * BLOCKSIZE:].reshape(WIDTH, 1)
            s = jnp.sum(W * U, axis=1)
            wt = W.T.astype(jnp.bfloat16)                         # [64, 128]
            wt2 = jnp.concatenate([wt, wt], axis=0)               # [128, 128]
            ubv = (U / WIDTH).astype(jnp.bfloat16)                # [128, 64]
            sbv = (s / WIDTH).astype(jnp.bfloat16)                # [128]
            neg_s0 = -jnp.sum(sbv.astype(jnp.float32))
            return (
                wt2,
                B,
                ubv,
                sbv.reshape(WIDTH, 1),
                jnp.full((128, 1), neg_s0, jnp.float32),
            )

        rep = P()
        _CACHE["hyper"] = jax.jit(shard_map(
            f, mesh=mesh,
            in_specs=(rep, rep, rep, P("c"), P("c")),
            out_specs=(rep, rep, rep, rep, rep),
            check_rep=False,
        ))
    out = _CACHE["hyper"](t, fc1_w, fc1_b, fc3_w, fc3_b)
    return [np.asarray(o) for o in out]


# ---------------------------------------------------------------------------
# entry point
# ---------------------------------------------------------------------------

def _install_ntff_hook():
    """antenv.axon_hooks (the hook registry) isn't shipped in this image;
    recreate it and wire the ctypes NTFF hook so trace=True works."""
    if _CACHE.get("ntff_hook_done"):
        return
    _CACHE["ntff_hook_done"] = True
    try:
        import types
        import antenv
        if "antenv.axon_hooks" not in sys.modules:
            mod = types.ModuleType("antenv.axon_hooks")
            mod._hook = None
            def set_axon_ntff_profile_hook(h):
                mod._hook = h
            def get_axon_ntff_profile_hook():
                return mod._hook
            mod.set_axon_ntff_profile_hook = set_axon_ntff_profile_hook
            mod.get_axon_ntff_profile_hook = get_axon_ntff_profile_hook
            sys.modules["antenv.axon_hooks"] = mod
            antenv.axon_hooks = mod
        from trn_agent_boot.trn_boot import _ntff_profile_via_ctypes
        hook = _ntff_profile_via_ctypes("/opt/axon/libaxon_pjrt.so")
        sys.modules["antenv.axon_hooks"].set_axon_ntff_profile_hook(hook)
    except Exception:
        pass


def kernel(t, z, logp_z, fc1_w, fc1_b, fc3_w, fc3_b, _trace=False):
    _enable_jax_cache()
    if _trace:
        _install_ntff_hook()
    from concourse.bass_utils import run_bass_kernel_spmd

    wt2, B, ub, sbv, neg_s0 = _hyper(
        np.asarray(t, np.float32), np.asarray(fc1_w, np.float32),
        np.asarray(fc1_b, np.float32), np.asarray(fc3_w, np.float32),
        np.asarray(fc3_b, np.float32),
    )

    if "nc" not in _CACHE:
        _CACHE["nc"] = _build_bass()
    nc = _CACHE["nc"]

    z = np.asarray(z, np.float32)
    common = dict(wt2=wt2, bias=B, ub=ub, sb=sbv, neg_s0=neg_s0)
    in_maps = [
        dict(z=z[i * BC:(i + 1) * BC], **common) for i in range(N_CORES)
    ]
    res = run_bass_kernel_spmd(nc, in_maps, list(range(N_CORES)), trace=_trace)
    dz = np.concatenate([res.results[i]["dz"] for i in range(N_CORES)], axis=0)
    dlogp = np.concatenate(
        [res.results[i]["dlogp"] for i in range(N_CORES)], axis=0
    )
    if _trace:
        kernel.last_exec_time_ns = res.exec_time_ns
        kernel.last_results = res
    return dz, dlogp
